# revision 68
# baseline (speedup 1.0000x reference)
"""GQA attention (B=2, L=2048, D=2048, H=16, KVH=4, Dh=128, RoPE, causal)
distributed over 8 TRN2 NeuronCores.

DP on batch (2) x TP on head-groups (4); the output projection is
query-sharded: per lq-block, cores exchange attention outputs with an
AllToAll (each core keeps its 128-query sub-slice of all 16 heads),
then computes out[128 q rows, all 2048 features] with the FULL Wo.

v3 changes over v2:
- A2A runs in two 4-rank replica groups ([[0-3],[4-7]]) with tight
  [NG*HD, P] buffers: half the wire of the padded 8-way A2A, and no
  zero-fill DMAs.
- finish(blk) (normalize + scatter + A2A trigger) emits eagerly right
  after attention(blk) instead of being paced into attention(blk+1),
  so the collective starts ~20-40us earlier and outproj never
  head-of-line blocks the PE queue.
- weight/x-tile loads are single 3D-AP DMAs (fewer descriptor issues
  on the sync/scalar queues).
"""
import sys
sys.path.insert(0, '/opt/trn_rl_repo')
import math
import numpy as np

B, L, D = 2, 2048, 2048
H, KVH, Dh = 16, 4, 128
HPC = H // KVH          # q heads per core = 4
NG = 4                  # TP group size
NC8 = 8
P = 128
LB = 512                # lq block size
NLB = L // LB           # 4
NKT = D // P            # 16 contraction tiles over D
NLT = L // P            # 16 tiles over L
HD = HPC * Dh           # 512
SCALE = 1.0 / math.sqrt(Dh)
NEG = -1.0e30

_cache = {}


def _emit(nc, tc, T):
    from concourse import mybir
    from concourse.bass import AP

    def dyn_half(ap, sel):
        """Shift a static AP by half the A2A buffer when sel==1 (runtime).

        One unconditional DMA with a per-core runtime offset replaces the
        cond= pair: a skipped cond-DMA on the gpsimd SWDGE queue bumps its
        completion semaphore without honoring its wait, which let consumers
        race the still-in-flight real transfer."""
        return AP(tensor=ap.tensor,
                  offset=ap.offset + sel * (NG * HD * P),
                  ap=ap.ap,
                  dep_tracking_offset=(ap.dep_tracking_offset
                                       if ap.dep_tracking_offset is not None
                                       else ap.offset))

    F32 = mybir.dt.float32
    SB = mybir.dt.bfloat16
    AF = mybir.ActivationFunctionType
    MULT = mybir.AluOpType.mult
    ADD = mybir.AluOpType.add

    xT = T["xT"]; cosT = T["cosT"]; sinT = T["sinT"]
    wq = T["wq"]; wk = T["wk"]; wv = T["wv"]; wo = T["wo"]
    rot = T["rot"]; iden = T["iden"]; sels = T["sels"]
    selr = T["selr"]; masks = T["masks"]; outD = T["outD"]

    def mmul(out, lhsT, rhs, **kw):
        nc.tensor.matmul(out, lhsT, rhs, skip_group_check=True, **kw)

    lp = nc.allow_low_precision(reason="bf16 attention pipeline")
    lp.__enter__()

    # batch id (0/1) of this core decides which A2A slots/rows are ours
    pid = nc.sync.partition_id()
    is_b0 = pid < NG
    is_b1 = pid >= NG
    # gpsimd-queue copy for the gt loads (regs are per-engine)
    gpid = nc.gpsimd.partition_id()
    g_is_b0 = gpid < NG
    g_is_b1 = gpid >= NG

    with tc.tile_pool(name="dram", bufs=1, space="DRAM") as dram:
        a2a_in = [dram.tile([NC8 * HD, P], SB, name=f"a2ain{j}")
                  for j in range(NLB)]
        a2a_out = [dram.tile([NC8 * HD, P], SB, name=f"a2aout{j}")
                   for j in range(NLB)]
        warm_in = dram.tile([NC8, P], SB, name="a2awarmin")
        warm_out = dram.tile([NC8, P], SB, name="a2awarmout")

        with tc.tile_pool(name="const", bufs=1) as cpool, \
             tc.tile_pool(name="acts", bufs=1) as apool, \
             tc.tile_pool(name="wopool", bufs=1) as wopool:
            rot_t = cpool.tile([Dh, Dh], SB)
            iden_t = cpool.tile([P, P], SB)
            sels_t = cpool.tile([P, HPC * HPC], SB)
            selr_t = cpool.tile([HPC, HPC * P], SB)
            mask_t = cpool.tile([P, P], SB)
            cos_t = cpool.tile([Dh, L], SB)
            sin_t = cpool.tile([Dh, L], SB)
            zt = cpool.tile([P, NKT * P], SB)
            nc.vector.memset(zt[:], 0.0)
            nc.sync.dma_start(rot_t[:], rot[:])
            nc.sync.dma_start(iden_t[:], iden[:])
            nc.sync.dma_start(sels_t[:], sels[:])
            nc.sync.dma_start(selr_t[:], selr[:])
            nc.sync.dma_start(mask_t[:], masks[:])
            # tiny warm-up A2A at kernel start: pays the first-collective
            # init + cross-core arrival sync under phase A, so A2A(0)
            # runs at steady-state latency
            nc.gpsimd.collective_compute(
                "AllToAll",
                mybir.AluOpType.bypass,
                replica_groups=[list(range(NC8))],
                ins=[warm_in[:]],
                outs=[warm_out[:]],
            )

            qkT = apool.tile([Dh, HPC + 1, L], SB)  # 4 q heads + k
            vnat = apool.tile([P, NLT, Dh], SB)     # v natural layout
            wo_t = wopool.tile([P, NKT, D], SB)     # full Wo (8.4MB)

            # ---------------- phase A: projections + RoPE ----------------
            with tc.tile_pool(name="wpool", bufs=1) as wpool, \
                 tc.tile_pool(name="xpool", bufs=1) as xpool, \
                 tc.tile_pool(name="ppsum", bufs=1, space="PSUM") as ppsum, \
                 tc.tile_pool(name="ptmp", bufs=1) as ptmp:
                wq_t = wpool.tile([P, NKT, HD], SB)
                wk_t = wpool.tile([P, NKT, Dh], SB)
                wv_t = wpool.tile([P, NKT, Dh], SB)
                # weights arrive host-pre-transposed to partition-major
                # [p, k, n] layouts: every DMA run is 2KB+ contiguous
                # per partition (the (k p) n layout's 1KB runs measured
                # only ~50GB/s and stalled the PE ~50us at kernel start)
                # k/v weights first (small; the k/v projections run
                # while the bulkier wq still streams), then wq chunks
                nc.scalar.dma_start(
                    wk_t[:], wk[:].rearrange("p (k n) -> p k n", n=Dh))
                nc.scalar.dma_start(
                    wv_t[:], wv[:].rearrange("p (k n) -> p k n", n=Dh))
                nc.scalar.dma_start(cos_t[:], cosT[:])
                nc.scalar.dma_start(sin_t[:], sinT[:])
                # wq chunks split across BOTH hwdge queues: on sync they
                # sit ahead of the xt3 block-1+ prefetch, so the 2MB wq
                # isn't starved by the x burst (chunk 0 measured landing
                # ~115us when scalar-only)
                wq_src = wq[:].rearrange("p (k n) -> p k n", n=HD)
                for ci, k4 in enumerate(range(0, NKT, 4)):
                    eng = nc.sync if ci % 2 == 0 else nc.scalar
                    eng.dma_start(wq_t[:, k4:k4 + 4, :],
                                  wq_src[:, k4:k4 + 4, :])
                # Wo streams one 2MB chunk per phase-A block iteration
                # (below): its 8MB would otherwise compete with wq/x for
                # DMA bandwidth exactly when the PE is starved for them
                wo_src = wo[:].rearrange("p (k n) -> p k n", n=D)

                vT_s = ptmp.tile([Dh, L], SB, name="vTs", bufs=1)
                for blk in range(NLB):
                    lqs = slice(blk * LB, (blk + 1) * LB)
                    xt3 = xpool.tile([P, NKT, LB], SB, name="xt3",
                                     tag="xt3", bufs=2)
                    # x is host-laid-out [p, blk, k, q]: a block's slab
                    # is contiguous per partition
                    xt_src = xT[:].rearrange("p (b k q) -> p b k q",
                                             b=NLB, k=NKT)
                    # all xt3 chunks on sync: a prefetch DMA that waits
                    # on buffer recycling must never sit in front of the
                    # PE-feeding qs copies on another queue
                    for k4 in range(0, NKT, 4):
                        nc.sync.dma_start(
                            xt3[:, k4:k4 + 4, :],
                            xt_src[:, blk, k4:k4 + 4, :])
                    if blk == 1:
                        # pre-zero the peer batch's A2A slots (finish()
                        # writes only this core's half; peers read zeros
                        # in the slots this core doesn't fill) — one
                        # 512KB DMA per lq-block
                        for zb in range(NLB):
                            dst0 = a2a_in[zb][0:NG * HD, :].rearrange(
                                "(o p) q -> p o q", p=P)
                            zsrc = zt[:].rearrange("p (o q) -> p o q", q=P)
                            nc.sync.dma_start(dyn_half(dst0, is_b0), zsrc)
                    def emit_rope(m, qs):
                        # RoPE: q' = cos*q + sin*(R q); emitted ~6 MMs
                        # after its projection group so the rot matmul
                        # never stalls the PE on the psum->sbuf qs copy
                        rq = ppsum.tile([Dh, LB], F32, name="rq",
                                        tag="rq", bufs=2)
                        nc.tensor.matmul(rq[:], rot_t[:], qs[:],
                                         start=True, stop=True)
                        # NOTE: keep RoPE off the gpsimd queue — gpsimd
                        # blocks inside collective triggers, and routing
                        # t1 there chained phase A to the warm A2A via
                        # qs-slot recycling (measured ~28us of PE stalls)
                        t1 = ptmp.tile([Dh, LB], SB, name="t1",
                                       tag="t1", bufs=2)
                        nc.vector.tensor_tensor(
                            out=t1[:], in0=qs[:],
                            in1=cos_t[:, lqs], op=MULT)
                        t2 = ptmp.tile([Dh, LB], SB, name="t2",
                                       tag="t2", bufs=2)
                        nc.vector.tensor_tensor(
                            out=t2[:], in0=rq[:],
                            in1=sin_t[:, lqs], op=MULT)
                        nc.vector.tensor_tensor(
                            out=qkT[:, m, lqs], in0=t1[:],
                            in1=t2[:], op=ADD)

                    # 6 projection outputs, k/v first: they only need
                    # the small wk/wv, so the PE starts while wq streams
                    ropeq = []
                    for m in [HPC, HPC + 1, 0, 1, 2, 3]:
                        pj = ppsum.tile([P, LB], F32, name="pj",
                                        tag="pj", bufs=3)
                        for kk in range(NKT):
                            if m < HPC:
                                wsl = wq_t[:, kk, m * Dh:(m + 1) * Dh]
                            elif m == HPC:
                                wsl = wk_t[:, kk, :]
                            else:
                                wsl = wv_t[:, kk, :]
                            nc.tensor.matmul(
                                pj[:], wsl, xt3[:, kk, :],
                                start=(kk == 0), stop=(kk == NKT - 1))
                            if kk == 6 and ropeq:
                                emit_rope(*ropeq.pop(0))
                        if m == HPC + 1:
                            # v: stage vT, XBAR DMA-transpose to natural
                            # layout (frees the PE + a psum bank)
                            nc.vector.tensor_copy(vT_s[:, lqs], pj[:])
                            for lt in range(blk * 4, blk * 4 + 4):
                                nc.sync.dma_start(
                                    vnat[:, lt, :],
                                    vT_s[:, lt * P:(lt + 1) * P],
                                    transpose=True)
                        else:
                            # qs on vector: the scalar queue's prefetch
                            # DMAs (which wait on PE progress) FIFO-
                            # blocked these copies for ~50us
                            qs = ptmp.tile([Dh, LB], SB, name="qs",
                                           tag="qs", bufs=3)
                            nc.vector.tensor_copy(qs[:], pj[:])
                            ropeq.append((m, qs))
                    while ropeq:
                        emit_rope(*ropeq.pop(0))

            # ------- phase B: attention + A2A + phase C: outproj -------
            with tc.tile_pool(name="bpsum", bufs=1, space="PSUM") as bpsum, \
                 tc.tile_pool(name="btile", bufs=1) as btile:

                # Wo streams at the START of attention: during phase A
                # its 8MB competed with the wq/x transfers the PE was
                # starved for (wq chunk 0 measured landing ~124us);
                # outproj(0) only needs it mid-attention(1)
                for k4 in range(0, NKT, 4):
                    nc.scalar.dma_start(wo_t[:, k4:k4 + 4, :],
                                        wo_src[:, k4:k4 + 4, :])

                # [p, slot(8), headslot(4), q] view of each a2a input
                a2a_slot = [t[:].rearrange("(s o p) q -> p s o q",
                                           s=NC8, o=HPC, p=P)
                            for t in a2a_in]

                HB = LB // 2

                # finish entries and the deferred trigger persist ACROSS
                # blocks: block b's last head-pair flushes during block
                # b+1's first tiles (then triggers A2A(b)), removing the
                # ~3.4us block-boundary PE stall on the reciprocal chain
                fin_pend = []   # (blk, h, ot, rinv2, row) awaiting rb/od
                trig_pend = []  # block whose A2A trigger awaits flushes

                def fin_flush():
                    eblk, hd, ot, rinv, row = fin_pend.pop(0)
                    # rb[p, q] = rinv[row, q] (partition broadcast
                    # via ones-row outer product)
                    sl = selr_t[0:2, row * P:(row + 1) * P]
                    rb = bpsum.tile([P, LB], F32, name="rb",
                                    tag="stx", bufs=2)
                    mmul(rb[:, 0:HB], sl,
                         rinv[0:2, 0:HB], start=True, stop=False)
                    mmul(rb[:, HB:], sl,
                         rinv[0:2, HB:], start=False, stop=True)
                    rbs = btile.tile([P, LB], SB, name="rbs",
                                     tag="rbs", bufs=2)
                    nc.scalar.copy(rbs[:], rb[:])
                    od = btile.tile([Dh, LB], SB, name="od",
                                    tag="od", bufs=2)
                    nc.vector.tensor_tensor(
                        out=od[:], in0=ot[:], in1=rbs[:], op=MULT)
                    # scatter od quarters to this batch's A2A slots:
                    # dst iterates (p, slot, q), src (p, j, q) —
                    # same flat order; the other half stays zeroed
                    src = od[:].rearrange("p (j q) -> p j q", q=P)
                    nc.sync.dma_start(
                        dyn_half(a2a_slot[eblk][:, 0:NG, hd, :], is_b1),
                        src)
                    if not fin_pend and trig_pend:
                        trigger(trig_pend.pop(0))

                def trigger(b):
                    nc.gpsimd.collective_compute(
                        "AllToAll",
                        mybir.AluOpType.bypass,
                        replica_groups=[list(range(NC8))],
                        ins=[a2a_in[b][:]],
                        outs=[a2a_out[b][:]],
                    )

                def attention(blk):
                    """Generator: attention for blk with the per-head
                    finish (normalize + scatter) software-pipelined one
                    key-tile into the following head; the previous
                    block's trailing pair flushes into this block's
                    first tiles."""
                    nlk = (blk + 1) * NG
                    hold = [None]   # even head's (ot) awaiting its pair
                    for h in range(HPC):
                        ot = bpsum.tile([Dh, LB], F32, name="ot",
                                        tag="ot", bufs=3)
                        racc = btile.tile([P, LB], SB, name="racc",
                                          tag="racc", bufs=2)
                        pend = None     # (pt, c0, i) waiting for PV

                        def pv(pt, c0, i, ot=None, racc=None):
                            mmul(ot[:, c0:], vnat[:, i, :], pt[:, c0:],
                                 start=(i == 0), stop=(i == nlk - 1))
                            if i > 0:
                                nc.vector.tensor_tensor(
                                    out=racc[:, c0:], in0=racc[:, c0:],
                                    in1=pt[:, c0:], op=ADD)

                        for i in range(nlk):
                            di = i - NG * blk
                            c0 = di * P if di > 0 else 0
                            st = bpsum.tile([P, LB], F32, name="st",
                                            tag="stx", bufs=2)
                            mmul(st[:, c0:],
                                 qkT[:, HPC, i * P:(i + 1) * P],
                                 qkT[:, h, blk * LB + c0:(blk + 1) * LB],
                                 start=True, stop=True)
                            if di >= 0:
                                nc.vector.tensor_tensor(
                                    out=st[:, c0:c0 + P],
                                    in0=st[:, c0:c0 + P],
                                    in1=mask_t[:], op=ADD)
                            # tile 0's exp lands directly in racc (it IS
                            # the running sum); later tiles go to pt
                            if i == 0:
                                pt = racc
                            else:
                                pt = btile.tile([P, LB], SB, name="pt",
                                                tag="pt", bufs=3)
                            nc.scalar.activation(pt[:, c0:], st[:, c0:],
                                                 AF.Exp, scale=SCALE)
                            # previous pair's finish, a couple of tiles
                            # deep so its rb never stalls the PE on the
                            # reciprocal latency
                            if i in (1, 2) and fin_pend:
                                fin_flush()
                            if pend is not None:
                                pv(*pend, ot=ot, racc=racc)
                            pend = (pt, c0, i)
                            yield
                        pv(*pend, ot=ot, racc=racc)
                        # denominator row h%2: rs2[h%2, :] = sum_p racc;
                        # heads finish in pairs so ONE reciprocal call
                        # (1.7us fixed cost on DVE) serves both
                        if h % 2 == 0:
                            rs2 = bpsum.tile([2, LB], F32, name="rs",
                                             tag="rs", bufs=1)
                            mmul(rs2[0:2, :], sels_t[:, 0:2], racc[:],
                                 start=True, stop=False)
                            hold[0] = (h, ot, rs2)
                        else:
                            hp, otp, rs2 = hold[0]
                            mmul(rs2[0:2, :], sels_t[:, 4:6], racc[:],
                                 start=False, stop=True)
                            rinv2 = btile.tile([2, LB], SB, name="ri",
                                               tag="ri", bufs=2)
                            nc.vector.reciprocal(rinv2[:], rs2[0:2, :])
                            fin_pend.append((blk, hp, otp, rinv2, 0))
                            fin_pend.append((blk, h, ot, rinv2, 1))
                        yield
                    if blk == NLB - 1:
                        # last block: flush inline and trigger now
                        while fin_pend:
                            fin_flush()
                        if trig_pend:
                            trigger(trig_pend.pop(0))
                        trigger(blk)
                    else:
                        # trailing pair flushes in the next block's first
                        # tiles; its A2A triggers right after
                        trig_pend.append(blk)

                def outproj(blk):
                    """Generator: emits outproj for blk (after its A2A),
                    yielding at interleave points. gt loads on the
                    gpsimd queue: it waits on the A2A there without
                    head-of-line blocking the sync queue's scatters."""
                    gt = btile.tile([P, NKT, P], SB, name="gt",
                                    tag="gt", bufs=2)
                    src = a2a_out[blk][:].rearrange(
                        "(k p) q -> p k q", p=P)
                    nc.gpsimd.dma_start(gt[:],
                                        dyn_half(src[:, 0:NKT, :], g_is_b1))
                    yield
                    for q in range(4):
                        fp = bpsum.tile([P, LB], F32, name="fp",
                                        tag="fp", bufs=2)
                        n0 = q * LB
                        for kk in range(NKT):
                            mmul(fp[:], gt[:, kk, :],
                                 wo_t[:, kk, n0:n0 + LB],
                                 start=(kk == 0), stop=(kk == NKT - 1))
                            if kk % 2 == 1:
                                yield
                        ft = btile.tile([P, LB], SB, name="ft",
                                        tag="ft", bufs=2)
                        nc.vector.tensor_copy(ft[:], fp[:])
                        nc.sync.dma_start(
                            outD[blk * P:(blk + 1) * P, n0:n0 + LB],
                            ft[:])
                        yield

                # driver: outproj(blk-1) (A2A-gated) interleaves into
                # attention(blk)'s yields starting min_iter in; the
                # finish work is inline in attention itself
                pending = []   # (min_iter, generator), driven in order
                for blk in range(NLB):
                    ag = attention(blk)
                    n_at = HPC * ((blk + 1) * NG + 1)    # attention yields
                    min_it = pending[0][0] if pending else 8
                    # last block drives much slower: leftover outproj(2)
                    # MMs (not A2A(3)-gated) then fill the PE during the
                    # tail A2A(3) wait — measured running empty at rate 20
                    rate = (10.0 if blk == NLB - 1 else 38.0) \
                        / max(n_at - min_it, 8)
                    credit = 1.0
                    for k, _ in enumerate(ag):
                        credit += rate
                        while credit >= 1.0 and pending \
                                and k >= pending[0][0]:
                            if next(pending[0][1], StopIteration) \
                                    is StopIteration:
                                pending.pop(0)
                            else:
                                credit -= 1.0
                    # first A2A pays cross-core arrival skew: gate its
                    # outproj deeper into the next attention block
                    pending.append((14 if blk == 0 else 8, outproj(blk)))
                for _, g in pending:
                    for _ in g:
                        pass
    lp.__exit__(None, None, None)


def _build():
    from concourse import bacc, tile, mybir

    F32 = mybir.dt.float32
    SB = mybir.dt.bfloat16
    nc = bacc.Bacc(None, target_bir_lowering=False, num_devices=NC8)

    T = {
        "xT": nc.declare_dram_parameter("xT", [P, NLB * NKT * LB], SB,
                                        isOutput=False),
        "cosT": nc.declare_dram_parameter("cosT", [Dh, L], SB, isOutput=False),
        "sinT": nc.declare_dram_parameter("sinT", [Dh, L], SB, isOutput=False),
        "wq": nc.declare_dram_parameter("wq", [P, NKT * HD], SB,
                                        isOutput=False),
        "wk": nc.declare_dram_parameter("wk", [P, NKT * Dh], SB,
                                        isOutput=False),
        "wv": nc.declare_dram_parameter("wv", [P, NKT * Dh], SB,
                                        isOutput=False),
        "wo": nc.declare_dram_parameter("wo", [P, NKT * D], SB,
                                        isOutput=False),
        "rot": nc.declare_dram_parameter("rot", [Dh, Dh], SB, isOutput=False),
        "iden": nc.declare_dram_parameter("iden", [P, P], SB, isOutput=False),
        "sels": nc.declare_dram_parameter("sels", [P, HPC * HPC], SB,
                                          isOutput=False),
        "selr": nc.declare_dram_parameter("selr", [HPC, HPC * P], SB,
                                          isOutput=False),
        "masks": nc.declare_dram_parameter("masks", [P, P], SB, isOutput=False),
        "outD": nc.declare_dram_parameter("outD", [NLB * P, D], SB,
                                          isOutput=True),
    }
    with tile.TileContext(nc) as tc, \
         nc.allow_low_precision(reason="bf16 attention pipeline"):
        _emit(nc, tc, T)
    return nc


def _prep(hidden_states, cos, sin, Wq, Wk, Wv, Wo):
    import ml_dtypes
    ndt = ml_dtypes.bfloat16

    rotm = np.zeros((Dh, Dh), dtype=np.float32)
    for p in range(Dh // 2):
        rotm[p, p + Dh // 2] = 1.0
        rotm[p + Dh // 2, p] = -1.0
    iden = np.eye(P, dtype=np.float32)
    # sels[:, 4h:4h+4] = e_h: column h all-ones -> rs4[h,:] = sum_p racc
    sels = np.zeros((P, HPC * HPC), dtype=np.float32)
    for h in range(HPC):
        sels[:, h * HPC + h] = 1.0
    # selr[:, h*128:(h+1)*128] = all-ones row h -> rb[p,:] = rinv4[h,:]
    selr = np.zeros((HPC, HPC * P), dtype=np.float32)
    for h in range(HPC):
        selr[h, h * P:(h + 1) * P] = 1.0
    # triangular tile mask: masked where kl > qq (S^T diagonal tile)
    kl = np.arange(P)[:, None]
    qq = np.arange(P)[None, :]
    masks = np.where(kl > qq, NEG, 0.0).astype(np.float32)

    def pmaj(w, n):
        # [D, n] -> partition-major [P, NKT * n]: row p holds k-tile-
        # ordered contiguous chunks, so DMAs stream 2KB+ runs
        return np.ascontiguousarray(
            w.reshape(NKT, P, n).transpose(1, 0, 2).reshape(P, NKT * n)
        ).astype(ndt)

    cosT = np.ascontiguousarray(cos.T).astype(ndt)
    sinT = np.ascontiguousarray(sin.T).astype(ndt)
    consts = {
        "rot": rotm.astype(ndt), "iden": iden.astype(ndt),
        "sels": sels.astype(ndt), "selr": selr.astype(ndt),
        "masks": masks.astype(ndt), "cosT": cosT, "sinT": sinT,
        "wo": pmaj(Wo, D),
    }
    maps = []
    for c in range(NC8):
        b, g = divmod(c, NG)
        xT = hidden_states[b].T            # [D, L]
        xh = np.ascontiguousarray(
            xT.reshape(NKT, P, NLB, LB).transpose(1, 2, 0, 3)
            .reshape(P, NLB * NKT * LB)).astype(ndt)
        maps.append(dict(
            consts,
            xT=xh,
            wq=pmaj(Wq[:, g * HD:(g + 1) * HD], HD),
            wk=pmaj(Wk[:, g * Dh:(g + 1) * Dh], Dh),
            wv=pmaj(Wv[:, g * Dh:(g + 1) * Dh], Dh),
        ))
    return maps


def kernel(hidden_states, cos, sin, Wq, Wk, Wv, Wo):
    from concourse.bass_utils import run_bass_kernel_spmd

    hidden_states = np.asarray(hidden_states, dtype=np.float32)
    cos = np.asarray(cos, dtype=np.float32)
    sin = np.asarray(sin, dtype=np.float32)
    Wq = np.asarray(Wq, dtype=np.float32)
    Wk = np.asarray(Wk, dtype=np.float32)
    Wv = np.asarray(Wv, dtype=np.float32)
    Wo = np.asarray(Wo, dtype=np.float32)

    if "nc" not in _cache:
        nc = _build()
        nc.finalize()
        _cache["nc"] = nc
    nc = _cache["nc"]
    in_maps = _prep(hidden_states, cos, sin, Wq, Wk, Wv, Wo)
    res = run_bass_kernel_spmd(nc, in_maps, list(range(NC8)))
    _cache["last_result"] = res
    out = np.empty((B, L, D), dtype=np.float32)
    for c in range(NC8):
        b, g = divmod(c, NG)
        od = np.asarray(res.results[c]["outD"], dtype=np.float32)
        for blk in range(NLB):
            out[b, blk * LB + g * P:blk * LB + (g + 1) * P, :] = \
                od[blk * P:(blk + 1) * P, :]
    return out



# revision 69
# speedup vs baseline: 1.1716x; 1.1716x over previous
"""GQA attention (B=2, L=2048, D=2048, H=16, KVH=4, Dh=128, RoPE, causal)
distributed over 8 TRN2 NeuronCores.

DP on batch (2) x TP on head-groups (4); the output projection is
query-sharded: per lq-block, cores exchange attention outputs with an
AllToAll (each core keeps its 128-query sub-slice of all 16 heads),
then computes out[128 q rows, all 2048 features] with the FULL Wo.

v3 changes over v2:
- A2A runs in two 4-rank replica groups ([[0-3],[4-7]]) with tight
  [NG*HD, P] buffers: half the wire of the padded 8-way A2A, and no
  zero-fill DMAs.
- finish(blk) (normalize + scatter + A2A trigger) emits eagerly right
  after attention(blk) instead of being paced into attention(blk+1),
  so the collective starts ~20-40us earlier and outproj never
  head-of-line blocks the PE queue.
- weight/x-tile loads are single 3D-AP DMAs (fewer descriptor issues
  on the sync/scalar queues).
"""
import sys
sys.path.insert(0, '/opt/trn_rl_repo')
import math
import numpy as np

B, L, D = 2, 2048, 2048
H, KVH, Dh = 16, 4, 128
HPC = H // KVH          # q heads per core = 4
NG = 4                  # TP group size
NC8 = 8
P = 128
LB = 512                # lq block size
NLB = L // LB           # 4
NKT = D // P            # 16 contraction tiles over D
NLT = L // P            # 16 tiles over L
HD = HPC * Dh           # 512
SCALE = 1.0 / math.sqrt(Dh)
NEG = -1.0e30

_cache = {}


def _emit(nc, tc, T):
    from concourse import mybir
    from concourse.bass import AP

    def dyn_half(ap, sel):
        """Shift a static AP by half the A2A buffer when sel==1 (runtime).

        One unconditional DMA with a per-core runtime offset replaces the
        cond= pair: a skipped cond-DMA on the gpsimd SWDGE queue bumps its
        completion semaphore without honoring its wait, which let consumers
        race the still-in-flight real transfer."""
        return AP(tensor=ap.tensor,
                  offset=ap.offset + sel * (NG * HD * P),
                  ap=ap.ap,
                  dep_tracking_offset=(ap.dep_tracking_offset
                                       if ap.dep_tracking_offset is not None
                                       else ap.offset))

    F32 = mybir.dt.float32
    SB = mybir.dt.bfloat16
    AF = mybir.ActivationFunctionType
    MULT = mybir.AluOpType.mult
    ADD = mybir.AluOpType.add

    xT = T["xT"]; cosT = T["cosT"]; sinT = T["sinT"]
    wq = T["wq"]; wk = T["wk"]; wv = T["wv"]; wo = T["wo"]
    rot = T["rot"]; iden = T["iden"]; sels = T["sels"]
    selr = T["selr"]; masks = T["masks"]; outD = T["outD"]

    def mmul(out, lhsT, rhs, **kw):
        nc.tensor.matmul(out, lhsT, rhs, skip_group_check=True, **kw)

    lp = nc.allow_low_precision(reason="bf16 attention pipeline")
    lp.__enter__()

    # batch id (0/1) of this core decides which A2A slots/rows are ours
    pid = nc.sync.partition_id()
    is_b0 = pid < NG
    is_b1 = pid >= NG
    # gpsimd-queue copy for the gt loads (regs are per-engine)
    gpid = nc.gpsimd.partition_id()
    g_is_b0 = gpid < NG
    g_is_b1 = gpid >= NG

    with tc.tile_pool(name="dram", bufs=1, space="DRAM") as dram:
        a2a_in = [dram.tile([NC8 * HD, P], SB, name=f"a2ain{j}")
                  for j in range(NLB)]
        a2a_out = [dram.tile([NC8 * HD, P], SB, name=f"a2aout{j}")
                   for j in range(NLB)]
        warm_in = dram.tile([NC8, P], SB, name="a2awarmin")
        warm_out = dram.tile([NC8, P], SB, name="a2awarmout")

        with tc.tile_pool(name="const", bufs=1) as cpool, \
             tc.tile_pool(name="acts", bufs=1) as apool, \
             tc.tile_pool(name="wopool", bufs=1) as wopool:
            rot_t = cpool.tile([Dh, Dh], SB)
            iden_t = cpool.tile([P, P], SB)
            sels_t = cpool.tile([P, HPC * HPC], SB)
            selr_t = cpool.tile([HPC, HPC * P], SB)
            mask_t = cpool.tile([P, P], SB)
            cos_t = cpool.tile([Dh, L], SB)
            sin_t = cpool.tile([Dh, L], SB)
            zt = cpool.tile([P, NKT * P], SB)
            nc.vector.memset(zt[:], 0.0)
            nc.sync.dma_start(rot_t[:], rot[:])
            nc.sync.dma_start(iden_t[:], iden[:])
            nc.sync.dma_start(sels_t[:], sels[:])
            nc.sync.dma_start(selr_t[:], selr[:])
            nc.sync.dma_start(mask_t[:], masks[:])
            # tiny warm-up A2A at kernel start: pays the first-collective
            # init + cross-core arrival sync under phase A, so A2A(0)
            # runs at steady-state latency
            nc.gpsimd.collective_compute(
                "AllToAll",
                mybir.AluOpType.bypass,
                replica_groups=[list(range(NC8))],
                ins=[warm_in[:]],
                outs=[warm_out[:]],
            )

            qkT = apool.tile([Dh, HPC + 1, L], SB)  # 4 q heads + k
            vnat = apool.tile([P, NLT, Dh], SB)     # v natural layout
            wo_t = wopool.tile([P, NKT, D], SB)     # full Wo (8.4MB)

            # ---------------- phase A: projections + RoPE ----------------
            with tc.tile_pool(name="wpool", bufs=1) as wpool, \
                 tc.tile_pool(name="xpool", bufs=1) as xpool, \
                 tc.tile_pool(name="ppsum", bufs=1, space="PSUM") as ppsum, \
                 tc.tile_pool(name="ptmp", bufs=1) as ptmp:
                wq_t = wpool.tile([P, NKT, HD], SB)
                wk_t = wpool.tile([P, NKT, Dh], SB)
                wv_t = wpool.tile([P, NKT, Dh], SB)
                # weights arrive host-pre-transposed to partition-major
                # [p, k, n] layouts: every DMA run is 2KB+ contiguous
                # per partition (the (k p) n layout's 1KB runs measured
                # only ~50GB/s and stalled the PE ~50us at kernel start)
                # k/v weights first (small; the k/v projections run
                # while the bulkier wq still streams), then wq chunks
                nc.scalar.dma_start(
                    wk_t[:], wk[:].rearrange("p (k n) -> p k n", n=Dh))
                nc.scalar.dma_start(
                    wv_t[:], wv[:].rearrange("p (k n) -> p k n", n=Dh))
                nc.scalar.dma_start(cos_t[:], cosT[:])
                nc.scalar.dma_start(sin_t[:], sinT[:])
                # wq chunks split across BOTH hwdge queues: on sync they
                # sit ahead of the xt3 block-1+ prefetch, so the 2MB wq
                # isn't starved by the x burst (chunk 0 measured landing
                # ~115us when scalar-only)
                wq_src = wq[:].rearrange("p (k n) -> p k n", n=HD)
                for ci, k4 in enumerate(range(0, NKT, 4)):
                    eng = nc.sync if ci % 2 == 0 else nc.scalar
                    eng.dma_start(wq_t[:, k4:k4 + 4, :],
                                  wq_src[:, k4:k4 + 4, :])
                # Wo streams one 2MB chunk per phase-A block iteration
                # (below): its 8MB would otherwise compete with wq/x for
                # DMA bandwidth exactly when the PE is starved for them
                wo_src = wo[:].rearrange("p (k n) -> p k n", n=D)

                vT_s = ptmp.tile([Dh, L], SB, name="vTs", bufs=1)
                for blk in range(NLB):
                    lqs = slice(blk * LB, (blk + 1) * LB)
                    xt3 = xpool.tile([P, NKT, LB], SB, name="xt3",
                                     tag="xt3", bufs=2)
                    # x is host-laid-out [p, blk, k, q]: a block's slab
                    # is contiguous per partition
                    xt_src = xT[:].rearrange("p (b k q) -> p b k q",
                                             b=NLB, k=NKT)
                    # all xt3 chunks on sync: a prefetch DMA that waits
                    # on buffer recycling must never sit in front of the
                    # PE-feeding qs copies on another queue
                    for k4 in range(0, NKT, 4):
                        nc.sync.dma_start(
                            xt3[:, k4:k4 + 4, :],
                            xt_src[:, blk, k4:k4 + 4, :])
                    if blk == 1:
                        # pre-zero the peer batch's A2A slots (finish()
                        # writes only this core's half; peers read zeros
                        # in the slots this core doesn't fill) — one
                        # 512KB DMA per lq-block
                        for zb in range(NLB):
                            dst0 = a2a_in[zb][0:NG * HD, :].rearrange(
                                "(o p) q -> p o q", p=P)
                            zsrc = zt[:].rearrange("p (o q) -> p o q", q=P)
                            nc.sync.dma_start(dyn_half(dst0, is_b0), zsrc)
                    def emit_rope(m, qs):
                        # RoPE: q' = cos*q + sin*(R q); emitted ~6 MMs
                        # after its projection group so the rot matmul
                        # never stalls the PE on the psum->sbuf qs copy
                        rq = ppsum.tile([Dh, LB], F32, name="rq",
                                        tag="rq", bufs=2)
                        nc.tensor.matmul(rq[:], rot_t[:], qs[:],
                                         start=True, stop=True)
                        # NOTE: keep RoPE off the gpsimd queue — gpsimd
                        # blocks inside collective triggers, and routing
                        # t1 there chained phase A to the warm A2A via
                        # qs-slot recycling (measured ~28us of PE stalls)
                        t1 = ptmp.tile([Dh, LB], SB, name="t1",
                                       tag="t1", bufs=2)
                        nc.vector.tensor_tensor(
                            out=t1[:], in0=qs[:],
                            in1=cos_t[:, lqs], op=MULT)
                        t2 = ptmp.tile([Dh, LB], SB, name="t2",
                                       tag="t2", bufs=2)
                        nc.vector.tensor_tensor(
                            out=t2[:], in0=rq[:],
                            in1=sin_t[:, lqs], op=MULT)
                        nc.vector.tensor_tensor(
                            out=qkT[:, m, lqs], in0=t1[:],
                            in1=t2[:], op=ADD)

                    # 6 projection outputs, k/v first: they only need
                    # the small wk/wv, so the PE starts while wq streams
                    ropeq = []
                    for m in [HPC, HPC + 1, 0, 1, 2, 3]:
                        pj = ppsum.tile([P, LB], F32, name="pj",
                                        tag="pj", bufs=3)
                        for kk in range(NKT):
                            if m < HPC:
                                wsl = wq_t[:, kk, m * Dh:(m + 1) * Dh]
                            elif m == HPC:
                                wsl = wk_t[:, kk, :]
                            else:
                                wsl = wv_t[:, kk, :]
                            nc.tensor.matmul(
                                pj[:], wsl, xt3[:, kk, :],
                                start=(kk == 0), stop=(kk == NKT - 1))
                            if kk == 6 and ropeq:
                                emit_rope(*ropeq.pop(0))
                        if m == HPC + 1:
                            # v: stage vT, XBAR DMA-transpose to natural
                            # layout (frees the PE + a psum bank)
                            nc.vector.tensor_copy(vT_s[:, lqs], pj[:])
                            for lt in range(blk * 4, blk * 4 + 4):
                                nc.sync.dma_start(
                                    vnat[:, lt, :],
                                    vT_s[:, lt * P:(lt + 1) * P],
                                    transpose=True)
                        else:
                            # qs on vector: the scalar queue's prefetch
                            # DMAs (which wait on PE progress) FIFO-
                            # blocked these copies for ~50us
                            qs = ptmp.tile([Dh, LB], SB, name="qs",
                                           tag="qs", bufs=3)
                            nc.vector.tensor_copy(qs[:], pj[:])
                            ropeq.append((m, qs))
                    while ropeq:
                        emit_rope(*ropeq.pop(0))

            # ------- phase B: attention + A2A + phase C: outproj -------
            with tc.tile_pool(name="bpsum", bufs=1, space="PSUM") as bpsum, \
                 tc.tile_pool(name="btile", bufs=1) as btile:

                # Wo streams at the START of attention: during phase A
                # its 8MB competed with the wq/x transfers the PE was
                # starved for (wq chunk 0 measured landing ~124us);
                # outproj(0) only needs it mid-attention(1)
                for k4 in range(0, NKT, 4):
                    nc.scalar.dma_start(wo_t[:, k4:k4 + 4, :],
                                        wo_src[:, k4:k4 + 4, :])

                # [p, slot(8), headslot(4), q] view of each a2a input
                a2a_slot = [t[:].rearrange("(s o p) q -> p s o q",
                                           s=NC8, o=HPC, p=P)
                            for t in a2a_in]

                HB = LB // 2

                # finish entries and the deferred trigger persist ACROSS
                # blocks: block b's last head-pair flushes during block
                # b+1's first tiles (then triggers A2A(b)), removing the
                # ~3.4us block-boundary PE stall on the reciprocal chain
                fin_pend = []   # (blk, h, ot, rinv2, row) awaiting rb/od
                trig_pend = []  # block whose A2A trigger awaits flushes

                def fin_flush():
                    eblk, hd, ot, rinv, row = fin_pend.pop(0)
                    # rb[p, q] = rinv[row, q] (partition broadcast
                    # via ones-row outer product)
                    sl = selr_t[0:2, row * P:(row + 1) * P]
                    rb = bpsum.tile([P, LB], F32, name="rb",
                                    tag="stx", bufs=2)
                    mmul(rb[:, 0:HB], sl,
                         rinv[0:2, 0:HB], start=True, stop=False)
                    mmul(rb[:, HB:], sl,
                         rinv[0:2, HB:], start=False, stop=True)
                    rbs = btile.tile([P, LB], SB, name="rbs",
                                     tag="rbs", bufs=2)
                    nc.scalar.copy(rbs[:], rb[:])
                    od = btile.tile([Dh, LB], SB, name="od",
                                    tag="od", bufs=2)
                    nc.vector.tensor_tensor(
                        out=od[:], in0=ot[:], in1=rbs[:], op=MULT)
                    # scatter od quarters to this batch's A2A slots:
                    # dst iterates (p, slot, q), src (p, j, q) —
                    # same flat order; the other half stays zeroed
                    src = od[:].rearrange("p (j q) -> p j q", q=P)
                    nc.sync.dma_start(
                        dyn_half(a2a_slot[eblk][:, 0:NG, hd, :], is_b1),
                        src)
                    if not fin_pend and trig_pend:
                        trigger(trig_pend.pop(0))

                def trigger(b):
                    nc.gpsimd.collective_compute(
                        "AllToAll",
                        mybir.AluOpType.bypass,
                        replica_groups=[list(range(NC8))],
                        ins=[a2a_in[b][:]],
                        outs=[a2a_out[b][:]],
                    )

                def attention(blk):
                    """Generator: attention for blk with the per-head
                    finish (normalize + scatter) software-pipelined one
                    key-tile into the following head; the previous
                    block's trailing pair flushes into this block's
                    first tiles."""
                    nlk = (blk + 1) * NG
                    hold = [None]   # even head's (ot) awaiting its pair
                    for h in range(HPC):
                        ot = bpsum.tile([Dh, LB], F32, name="ot",
                                        tag="ot", bufs=3)
                        racc = btile.tile([P, LB], SB, name="racc",
                                          tag="racc", bufs=2)
                        pend = None     # (pt, c0, i) waiting for PV

                        def pv(pt, c0, i, ot=None, racc=None):
                            mmul(ot[:, c0:], vnat[:, i, :], pt[:, c0:],
                                 start=(i == 0), stop=(i == nlk - 1))
                            if i > 0:
                                nc.vector.tensor_tensor(
                                    out=racc[:, c0:], in0=racc[:, c0:],
                                    in1=pt[:, c0:], op=ADD)

                        for i in range(nlk):
                            di = i - NG * blk
                            c0 = di * P if di > 0 else 0
                            st = bpsum.tile([P, LB], F32, name="st",
                                            tag="stx", bufs=2)
                            mmul(st[:, c0:],
                                 qkT[:, HPC, i * P:(i + 1) * P],
                                 qkT[:, h, blk * LB + c0:(blk + 1) * LB],
                                 start=True, stop=True)
                            if di >= 0:
                                nc.vector.tensor_tensor(
                                    out=st[:, c0:c0 + P],
                                    in0=st[:, c0:c0 + P],
                                    in1=mask_t[:], op=ADD)
                            # tile 0's exp lands directly in racc (it IS
                            # the running sum); later tiles go to pt
                            if i == 0:
                                pt = racc
                            else:
                                pt = btile.tile([P, LB], SB, name="pt",
                                                tag="pt", bufs=3)
                            nc.scalar.activation(pt[:, c0:], st[:, c0:],
                                                 AF.Exp, scale=SCALE)
                            # previous pair's finish, a couple of tiles
                            # deep so its rb never stalls the PE on the
                            # reciprocal latency
                            if i in (1, 2) and fin_pend:
                                fin_flush()
                            if pend is not None:
                                pv(*pend, ot=ot, racc=racc)
                            pend = (pt, c0, i)
                            yield
                        pv(*pend, ot=ot, racc=racc)
                        # denominator row h%2: rs2[h%2, :] = sum_p racc;
                        # heads finish in pairs so ONE reciprocal call
                        # (1.7us fixed cost on DVE) serves both
                        if h % 2 == 0:
                            rs2 = bpsum.tile([2, LB], F32, name="rs",
                                             tag="rs", bufs=1)
                            mmul(rs2[0:2, :], sels_t[:, 0:2], racc[:],
                                 start=True, stop=False)
                            hold[0] = (h, ot, rs2)
                        else:
                            hp, otp, rs2 = hold[0]
                            mmul(rs2[0:2, :], sels_t[:, 4:6], racc[:],
                                 start=False, stop=True)
                            rinv2 = btile.tile([2, LB], SB, name="ri",
                                               tag="ri", bufs=2)
                            nc.vector.reciprocal(rinv2[:], rs2[0:2, :])
                            fin_pend.append((blk, hp, otp, rinv2, 0))
                            fin_pend.append((blk, h, ot, rinv2, 1))
                        yield
                    if blk == NLB - 1:
                        # last block: flush inline and trigger now
                        while fin_pend:
                            fin_flush()
                        if trig_pend:
                            trigger(trig_pend.pop(0))
                        trigger(blk)
                    else:
                        # trailing pair flushes in the next block's first
                        # tiles; its A2A triggers right after
                        trig_pend.append(blk)

                def outproj(blk):
                    """Generator: emits outproj for blk (after its A2A),
                    yielding at interleave points. gt loads on the
                    gpsimd queue: it waits on the A2A there without
                    head-of-line blocking the sync queue's scatters."""
                    gt = btile.tile([P, NKT, P], SB, name="gt",
                                    tag="gt", bufs=2)
                    src = a2a_out[blk][:].rearrange(
                        "(k p) q -> p k q", p=P)
                    nc.gpsimd.dma_start(gt[:],
                                        dyn_half(src[:, 0:NKT, :], g_is_b1))
                    yield
                    for q in range(4):
                        fp = bpsum.tile([P, LB], F32, name="fp",
                                        tag="fp", bufs=2)
                        n0 = q * LB
                        for kk in range(NKT):
                            mmul(fp[:], gt[:, kk, :],
                                 wo_t[:, kk, n0:n0 + LB],
                                 start=(kk == 0), stop=(kk == NKT - 1))
                            if kk % 2 == 1:
                                yield
                        ft = btile.tile([P, LB], SB, name="ft",
                                        tag="ft", bufs=2)
                        nc.vector.tensor_copy(ft[:], fp[:])
                        nc.sync.dma_start(
                            outD[blk * P:(blk + 1) * P, n0:n0 + LB],
                            ft[:])
                        yield

                # driver: outproj(blk-1) (A2A-gated) interleaves into
                # attention(blk)'s yields starting min_iter in; the
                # finish work is inline in attention itself
                pending = []   # (min_iter, generator), driven in order
                for blk in range(NLB):
                    ag = attention(blk)
                    n_at = HPC * ((blk + 1) * NG + 1)    # attention yields
                    min_it = pending[0][0] if pending else 8
                    # uniform drive rate: starving the last block (rate
                    # 10) meant outproj(2) was never driven during
                    # attention(3), so its gt DMA emitted only in the
                    # tail — on the gpsimd queue AFTER trigger(3)'s
                    # inline completion wait (measured 24us stall)
                    rate = 38.0 / max(n_at - min_it, 8)
                    credit = 1.0
                    for k, _ in enumerate(ag):
                        credit += rate
                        while credit >= 1.0 and pending \
                                and k >= pending[0][0]:
                            if next(pending[0][1], StopIteration) \
                                    is StopIteration:
                                pending.pop(0)
                            else:
                                credit -= 1.0
                    # first A2A pays cross-core arrival skew: gate its
                    # outproj deeper into the next attention block
                    pending.append((14 if blk == 0 else 8, outproj(blk)))
                for _, g in pending:
                    for _ in g:
                        pass
    lp.__exit__(None, None, None)


def _build():
    from concourse import bacc, tile, mybir

    F32 = mybir.dt.float32
    SB = mybir.dt.bfloat16
    nc = bacc.Bacc(None, target_bir_lowering=False, num_devices=NC8)

    T = {
        "xT": nc.declare_dram_parameter("xT", [P, NLB * NKT * LB], SB,
                                        isOutput=False),
        "cosT": nc.declare_dram_parameter("cosT", [Dh, L], SB, isOutput=False),
        "sinT": nc.declare_dram_parameter("sinT", [Dh, L], SB, isOutput=False),
        "wq": nc.declare_dram_parameter("wq", [P, NKT * HD], SB,
                                        isOutput=False),
        "wk": nc.declare_dram_parameter("wk", [P, NKT * Dh], SB,
                                        isOutput=False),
        "wv": nc.declare_dram_parameter("wv", [P, NKT * Dh], SB,
                                        isOutput=False),
        "wo": nc.declare_dram_parameter("wo", [P, NKT * D], SB,
                                        isOutput=False),
        "rot": nc.declare_dram_parameter("rot", [Dh, Dh], SB, isOutput=False),
        "iden": nc.declare_dram_parameter("iden", [P, P], SB, isOutput=False),
        "sels": nc.declare_dram_parameter("sels", [P, HPC * HPC], SB,
                                          isOutput=False),
        "selr": nc.declare_dram_parameter("selr", [HPC, HPC * P], SB,
                                          isOutput=False),
        "masks": nc.declare_dram_parameter("masks", [P, P], SB, isOutput=False),
        "outD": nc.declare_dram_parameter("outD", [NLB * P, D], SB,
                                          isOutput=True),
    }
    with tile.TileContext(nc) as tc, \
         nc.allow_low_precision(reason="bf16 attention pipeline"):
        _emit(nc, tc, T)
    return nc


def _prep(hidden_states, cos, sin, Wq, Wk, Wv, Wo):
    import ml_dtypes
    ndt = ml_dtypes.bfloat16

    rotm = np.zeros((Dh, Dh), dtype=np.float32)
    for p in range(Dh // 2):
        rotm[p, p + Dh // 2] = 1.0
        rotm[p + Dh // 2, p] = -1.0
    iden = np.eye(P, dtype=np.float32)
    # sels[:, 4h:4h+4] = e_h: column h all-ones -> rs4[h,:] = sum_p racc
    sels = np.zeros((P, HPC * HPC), dtype=np.float32)
    for h in range(HPC):
        sels[:, h * HPC + h] = 1.0
    # selr[:, h*128:(h+1)*128] = all-ones row h -> rb[p,:] = rinv4[h,:]
    selr = np.zeros((HPC, HPC * P), dtype=np.float32)
    for h in range(HPC):
        selr[h, h * P:(h + 1) * P] = 1.0
    # triangular tile mask: masked where kl > qq (S^T diagonal tile)
    kl = np.arange(P)[:, None]
    qq = np.arange(P)[None, :]
    masks = np.where(kl > qq, NEG, 0.0).astype(np.float32)

    def pmaj(w, n):
        # [D, n] -> partition-major [P, NKT * n]: row p holds k-tile-
        # ordered contiguous chunks, so DMAs stream 2KB+ runs
        return np.ascontiguousarray(
            w.reshape(NKT, P, n).transpose(1, 0, 2).reshape(P, NKT * n)
        ).astype(ndt)

    cosT = np.ascontiguousarray(cos.T).astype(ndt)
    sinT = np.ascontiguousarray(sin.T).astype(ndt)
    consts = {
        "rot": rotm.astype(ndt), "iden": iden.astype(ndt),
        "sels": sels.astype(ndt), "selr": selr.astype(ndt),
        "masks": masks.astype(ndt), "cosT": cosT, "sinT": sinT,
        "wo": pmaj(Wo, D),
    }
    maps = []
    for c in range(NC8):
        b, g = divmod(c, NG)
        xT = hidden_states[b].T            # [D, L]
        xh = np.ascontiguousarray(
            xT.reshape(NKT, P, NLB, LB).transpose(1, 2, 0, 3)
            .reshape(P, NLB * NKT * LB)).astype(ndt)
        maps.append(dict(
            consts,
            xT=xh,
            wq=pmaj(Wq[:, g * HD:(g + 1) * HD], HD),
            wk=pmaj(Wk[:, g * Dh:(g + 1) * Dh], Dh),
            wv=pmaj(Wv[:, g * Dh:(g + 1) * Dh], Dh),
        ))
    return maps


def kernel(hidden_states, cos, sin, Wq, Wk, Wv, Wo):
    from concourse.bass_utils import run_bass_kernel_spmd

    hidden_states = np.asarray(hidden_states, dtype=np.float32)
    cos = np.asarray(cos, dtype=np.float32)
    sin = np.asarray(sin, dtype=np.float32)
    Wq = np.asarray(Wq, dtype=np.float32)
    Wk = np.asarray(Wk, dtype=np.float32)
    Wv = np.asarray(Wv, dtype=np.float32)
    Wo = np.asarray(Wo, dtype=np.float32)

    if "nc" not in _cache:
        nc = _build()
        nc.finalize()
        _cache["nc"] = nc
    nc = _cache["nc"]
    in_maps = _prep(hidden_states, cos, sin, Wq, Wk, Wv, Wo)
    res = run_bass_kernel_spmd(nc, in_maps, list(range(NC8)))
    _cache["last_result"] = res
    out = np.empty((B, L, D), dtype=np.float32)
    for c in range(NC8):
        b, g = divmod(c, NG)
        od = np.asarray(res.results[c]["outD"], dtype=np.float32)
        for blk in range(NLB):
            out[b, blk * LB + g * P:blk * LB + (g + 1) * P, :] = \
                od[blk * P:(blk + 1) * P, :]
    return out

